# revision 5
# baseline (speedup 1.0000x reference)
"""Euler-Maruyama SDE paths on Trainium2 (Bass/Tile, 8 NeuronCores).

Recurrence: Z[:, t] = Z[:, t-1] * (1 + r*dt + s*sqrt(dt)*W[:, t]), Z[:, 0] = Z0.
Purely multiplicative per step, so it maps onto the DVE tensor_tensor_scan
instruction (op0=mult, op1=bypass): one scan per batch row along the time axis.

Sharding: batch (path) dim split evenly across the 8 cores (pure data
parallel); the time recurrence stays on-core; weights are baked as immediates.

Per-core layout: rows -> [128 partitions x R rows-per-partition x G tiles],
row = p*(R*G) + t*R + j.  Per tile: DMA W rows in, scalar-engine affine
(M = scale*W + bias, in place), vector-engine scan with initial=Z0 column,
DMA Z rows out.
"""

import numpy as np

import concourse.bacc as bacc
import concourse.bass as bass
import concourse.mybir as mybir
import concourse.tile as tile
from concourse.bass_utils import run_bass_kernel_spmd

N_CORES = 8
B = 131072
NT = 1024  # time steps; output has NT+1 columns
ROWS = B // N_CORES  # 16384 rows per core
P = 128  # SBUF partitions
R = 2  # rows per partition per tile
G = ROWS // (P * R)  # tiles per core

F32 = mybir.dt.float32


def _build_nc(rows: int, nt: int, r: float, s: float, rpp: int):
    """Build the per-core Bass program. rows = batch rows on this core,
    nt = time steps, rpp = rows per partition per tile."""
    dt = np.float32(1.0 / nt)
    sdt = np.float32(np.sqrt(dt))
    scale = float(np.float32(s) * sdt)  # multiplies W
    bias = float(np.float32(1.0) + np.float32(r) * dt)

    g = rows // (P * rpp)
    assert rows == P * rpp * g

    nc = bacc.Bacc("TRN2", target_bir_lowering=False, debug=False,
                   num_devices=N_CORES)
    W = nc.dram_tensor("W", [rows, nt + 1], F32, kind="ExternalInput").ap()
    Z0 = nc.dram_tensor("Z0", [rows], F32, kind="ExternalInput").ap()
    Z = nc.dram_tensor("Z", [rows, nt + 1], F32, kind="ExternalOutput").ap()

    # row = p*(rpp*g) + t*rpp + j
    W_v = W.rearrange("(p t j) c -> p t j c", p=P, t=g, j=rpp)
    Z_v = Z.rearrange("(p t j) c -> p t j c", p=P, t=g, j=rpp)
    Z0_v = Z0.rearrange("(p m) -> p m", p=P)  # [P, rpp*g], col m = t*rpp + j

    with tile.TileContext(nc) as tc:
        with (
            tc.tile_pool(name="z0", bufs=1) as z0_pool,
            tc.tile_pool(name="w", bufs=4) as w_pool,
            tc.tile_pool(name="o", bufs=4) as o_pool,
        ):
            z0_all = z0_pool.tile([P, rpp * g], F32)
            nc.sync.dma_start(z0_all[:], Z0_v[:])
            bias_t = z0_pool.tile([P, 1], F32, tag="bias")
            nc.vector.memset(bias_t[:], bias)

            for t in range(g):
                wt = w_pool.tile([P, rpp, nt], F32, tag="w")
                ot = o_pool.tile([P, rpp, nt + 1], F32, tag="o")
                # load W[:, 1:] for this tile's rows
                nc.sync.dma_start(wt[:], W_v[:, t, :, 1:])
                # M = scale*W + bias, in place (ACT engine)
                nc.scalar.activation(
                    wt[:], wt[:], mybir.ActivationFunctionType.Identity,
                    bias=bias_t[:], scale=scale,
                )
                # Z[:, 0] = Z0
                nc.gpsimd.tensor_copy(ot[:, :, 0], z0_all[:, t * rpp:(t + 1) * rpp])
                # Z[:, 1:] = cumprod(M) * Z0 via DVE scan
                for j in range(rpp):
                    nc.vector.tensor_tensor_scan(
                        out=ot[:, j, 1:],
                        data0=wt[:, j, :],
                        data1=wt[:, j, :],
                        initial=z0_all[:, t * rpp + j: t * rpp + j + 1],
                        op0=mybir.AluOpType.mult,
                        op1=mybir.AluOpType.bypass,
                    )
                nc.sync.dma_start(Z_v[:, t, :, :], ot[:])

    nc.compile()
    return nc


_NC_CACHE: dict = {}


def _get_nc(r: float, s: float):
    key = (r, s)
    if key not in _NC_CACHE:
        _NC_CACHE[key] = _build_nc(ROWS, NT, r, s, R)
    return _NC_CACHE[key]


def run(Z0, W, Wf, Wg, trace=False, tmpdir=None):
    Z0 = np.ascontiguousarray(np.asarray(Z0, dtype=np.float32))
    W = np.ascontiguousarray(np.asarray(W, dtype=np.float32))
    r = float(np.asarray(Wf, dtype=np.float32)[0, 0])
    s = float(np.asarray(Wg, dtype=np.float32)[0, 0])
    nc = _get_nc(r, s)

    in_maps = [
        {"W": W[c * ROWS:(c + 1) * ROWS], "Z0": Z0[c * ROWS:(c + 1) * ROWS]}
        for c in range(N_CORES)
    ]
    res = run_bass_kernel_spmd(nc, in_maps, list(range(N_CORES)), trace=trace,
                               tmpdir=tmpdir)
    Z = np.concatenate([res.results[c]["Z"] for c in range(N_CORES)], axis=0)
    return (Z, W), res


def kernel(Z0, W, Wf, Wg):
    (Z, W_out), _ = run(Z0, W, Wf, Wg, trace=False)
    return Z, W_out


# revision 7
# speedup vs baseline: 1.4532x; 1.4532x over previous
"""Euler-Maruyama SDE paths on Trainium2 (Bass/Tile, 8 NeuronCores).

Recurrence: Z[:, t] = Z[:, t-1] * (1 + r*dt + s*sqrt(dt)*W[:, t]), Z[:, 0] = Z0.
Purely multiplicative per step, so it maps onto the DVE tensor_tensor_scan
instruction (op0=mult, op1=bypass): one scan per batch row along the time axis.

Sharding: batch (path) dim split evenly across the 8 cores (pure data
parallel); the time recurrence stays on-core; weights are baked as immediates.

Per-core layout: rows -> [128 partitions x R rows-per-partition x G tiles],
row = p*(R*G) + t*R + j.  Per tile: DMA W rows in, scalar-engine affine
(M = scale*W + bias, in place), vector-engine scan with initial=Z0 column,
DMA Z rows out.
"""

import numpy as np

import concourse.bacc as bacc
import concourse.bass as bass
import concourse.mybir as mybir
import concourse.tile as tile
from concourse.bass_utils import run_bass_kernel_spmd

N_CORES = 8
B = 131072
NT = 1024  # time steps; output has NT+1 columns
ROWS = B // N_CORES  # 16384 rows per core
P = 128  # SBUF partitions
R = 2  # rows per partition per tile
G = ROWS // (P * R)  # tiles per core

F32 = mybir.dt.float32


def _build_nc(rows: int, nt: int, r: float, s: float, rpp: int):
    """Build the per-core Bass program. rows = batch rows on this core,
    nt = time steps, rpp = rows per partition per tile."""
    dt = np.float32(1.0 / nt)
    sdt = np.float32(np.sqrt(dt))
    scale = float(np.float32(s) * sdt)  # multiplies W
    bias = float(np.float32(1.0) + np.float32(r) * dt)

    g = rows // (P * rpp)
    assert rows == P * rpp * g

    nc = bacc.Bacc("TRN2", target_bir_lowering=False, debug=False,
                   num_devices=N_CORES)
    W = nc.dram_tensor("W", [rows, nt + 1], F32, kind="ExternalInput").ap()
    Z0 = nc.dram_tensor("Z0", [rows], F32, kind="ExternalInput").ap()
    Z = nc.dram_tensor("Z", [rows, nt + 1], F32, kind="ExternalOutput").ap()

    # row = p*(rpp*g) + t*rpp + j
    W_v = W.rearrange("(p t j) c -> p t j c", p=P, t=g, j=rpp)
    Z_v = Z.rearrange("(p t j) c -> p t j c", p=P, t=g, j=rpp)
    Z0_v = Z0.rearrange("(p m) -> p m", p=P)  # [P, rpp*g], col m = t*rpp + j

    with tile.TileContext(nc) as tc:
        with (
            tc.tile_pool(name="z0", bufs=1) as z0_pool,
            tc.tile_pool(name="w", bufs=6) as w_pool,
            tc.tile_pool(name="o", bufs=6) as o_pool,
        ):
            z0_all = z0_pool.tile([P, rpp * g], F32)
            nc.sync.dma_start(z0_all[:], Z0_v[:])
            bias_t = z0_pool.tile([P, 1], F32, tag="bias")
            nc.vector.memset(bias_t[:], bias)

            for t in range(g):
                # wt column 0 is a constant 1.0 so the scan emits Z0 as
                # output column 0; columns 1.. hold M = scale*W + bias.
                wt = w_pool.tile([P, rpp, nt + 1], F32, tag="w")
                ot = o_pool.tile([P, rpp, nt + 1], F32, tag="o")
                # load W[:, 1:] for this tile's rows (in-DMAs issue on sync)
                nc.sync.dma_start(wt[:, :, 1:], W_v[:, t, :, 1:])
                nc.vector.memset(wt[:, :, 0:1], 1.0)
                # M = scale*W + bias, in place (ACT engine)
                nc.scalar.activation(
                    wt[:, :, 1:], wt[:, :, 1:],
                    mybir.ActivationFunctionType.Identity,
                    bias=bias_t[:], scale=scale,
                )
                # Z row = scan([1|M], init=Z0): out[0]=Z0, out[t]=cumprod*Z0
                for j in range(rpp):
                    nc.vector.tensor_tensor_scan(
                        out=ot[:, j, :],
                        data0=wt[:, j, :],
                        data1=wt[:, j, :],
                        initial=z0_all[:, t * rpp + j: t * rpp + j + 1],
                        op0=mybir.AluOpType.mult,
                        op1=mybir.AluOpType.bypass,
                    )
                # out-DMAs issue on the gpsimd sequencer so they never
                # block in-DMA prefetch on sync
                nc.gpsimd.dma_start(Z_v[:, t, :, :], ot[:])

    nc.compile()
    return nc


_NC_CACHE: dict = {}


def _get_nc(r: float, s: float):
    key = (r, s)
    if key not in _NC_CACHE:
        _NC_CACHE[key] = _build_nc(ROWS, NT, r, s, R)
    return _NC_CACHE[key]


def run(Z0, W, Wf, Wg, trace=False, tmpdir=None):
    Z0 = np.ascontiguousarray(np.asarray(Z0, dtype=np.float32))
    W = np.ascontiguousarray(np.asarray(W, dtype=np.float32))
    r = float(np.asarray(Wf, dtype=np.float32)[0, 0])
    s = float(np.asarray(Wg, dtype=np.float32)[0, 0])
    nc = _get_nc(r, s)

    in_maps = [
        {"W": W[c * ROWS:(c + 1) * ROWS], "Z0": Z0[c * ROWS:(c + 1) * ROWS]}
        for c in range(N_CORES)
    ]
    res = run_bass_kernel_spmd(nc, in_maps, list(range(N_CORES)), trace=trace,
                               tmpdir=tmpdir)
    Z = np.concatenate([res.results[c]["Z"] for c in range(N_CORES)], axis=0)
    return (Z, W), res


def kernel(Z0, W, Wf, Wg):
    (Z, W_out), _ = run(Z0, W, Wf, Wg, trace=False)
    return Z, W_out
